# revision 64
# baseline (speedup 1.0000x reference)
"""MQA attention (16 Q heads, 1 KV head) on 8 trn2 NeuronCores.

Sharding: data-parallel on batch (2) x tensor-parallel on Q heads (4 per
core). Each core computes K/V for its batch, attention for its 4 heads,
and a row-parallel o_proj partial; the host sums the 4 partials per batch.

Per-core layout strategy: all matmul contractions on partitions, all
matmul operands bf16.
  front end: inputs land as a few big partition-major DMAs (4-32KB
     contiguous runs per partition) with critical-path priority - wkv
     first on the gpsimd ring, xt0 halves on the scalar ring, wq first
     on the sync ring, the wo bulk last; HAM-warmup junk matmuls on the
     early identity tile keep the PE active during the DMA wait so the
     real chains run at 2.4 GHz from the first tile
  kv proj packed into one chain (k -> PSUM rows 0:64, v -> 64:128); k
     duplicated to partitions 64:128 by one SBUF-SBUF DMA per chunk so
     the two row-tiled score matmuls see k on both halves; v transposed
     to [keys, dim] blocks by PE transposes from partitions 64:128
  qT = wqT.T @ xT -> [256, 2048] as 2 head-pair tiles [128, 2048]
  scoresT [k, q] per (pair, qchunk, kblock) in PSUM [128, 1024]: two
     K=64 matmuls on disjoint partition halves (head-even on array rows
     0:63, head-odd on 64:127) which partially overlap on the PE
  exp: ScalarE table-exp with per-key bias for 11/16 key blocks; the
     other 5 on DVE via the Schraudolph bit-trick (i16 = a*s + b,
     bitcast to bf16, ~2% rms) so the ACT engine keeps pace
  PV: lhsT = [v | ones] bf16 [128, 65] -> attn_outT [64, q] + denom row
  normalize: reciprocal + DRAM-bounce partition-broadcast + DVE multiply,
     staged across later units so the in-order DVE queue never blocks
  o_proj: out[q, hidden] partial = attnT.T @ woT (bf16), K=256, as a
     separate phase; output DMAs alternate scalar/gpsimd rings keeping
     sync clear for the last qchunk's norm round-trip
"""
import sys

sys.path.insert(0, "/opt/trn_rl_repo")

import ml_dtypes
import numpy as np

import concourse.bass as bass
import concourse.bacc as bacc
import concourse.tile as tile
from concourse import mybir
from concourse.bass_utils import run_bass_kernel_spmd
from concourse.tile_rust import add_dep_helper

HIDDEN = 1024
NH = 16
D = 64
B = 2
S = 2048
NCORES = 8
HEADS_PER_CORE = 4
KB = S // 128   # 16 key blocks
QC = S // 512   # 4 query chunks
P = 128

F32 = mybir.dt.float32
F32R = mybir.dt.float32r
BF16 = mybir.dt.bfloat16
I16 = mybir.dt.int16

DVE_KBS = (2, 5, 8, 11, 14)
SCH_A = 128.0 / float(np.log(2.0))
SCH_B = 16256.0 - 0.0397 * SCH_A + 0.5

_CACHE = {}


def build_kernel():
    nc = bacc.Bacc("TRN2", target_bir_lowering=False, debug=False,
                   num_devices=NCORES)

    xT = nc.dram_tensor("xT", [P, QC, 8, 512], BF16, kind="ExternalInput")
    wqT = nc.dram_tensor("wqT", [P, 8, 256], BF16, kind="ExternalInput")
    wkvT = nc.dram_tensor("wkvT", [P, 8, 128], BF16, kind="ExternalInput")
    identT = nc.dram_tensor("identT", [P, P], BF16, kind="ExternalInput")
    woT = nc.dram_tensor("woT", [P, 2, HIDDEN], BF16, kind="ExternalInput")
    bias2d = nc.dram_tensor("bias2d", [P, KB], F32, kind="ExternalInput")
    bdve = nc.dram_tensor("bdve", [P, KB], F32, kind="ExternalInput")
    ones2d = nc.dram_tensor("ones2d", [P, KB], BF16, kind="ExternalInput")
    out = nc.dram_tensor("out", [S, HIDDEN], BF16, kind="ExternalOutput")
    bounce = nc.dram_tensor("bounce", [QC, 2, 2, 512], F32)

    with tile.TileContext(nc) as tc:
        with tc.tile_pool(name="persist", bufs=1) as persist:
            xts = [persist.tile([P, 8, 512], BF16, name=f"xt{jj}")
                   for jj in range(QC)]  # per-qchunk xT tiles
            qt = persist.tile([P, 2, S], BF16)          # qT head pairs
            kt = persist.tile([P, S], BF16)             # kT dup both halves
            vaug = persist.tile([P, KB, D + 1], BF16)   # [v | ones]
            attnT_js = [persist.tile([P, 2, 512], BF16, name=f"attnT{jj}")
                        for jj in range(QC)]  # per-j normalized attnT
            wq_sb = persist.tile([P, 8, 256], BF16)
            wkv_sb = persist.tile([P, 8, 128], BF16)
            vt_sb = persist.tile([P, S], BF16)
            id_sb = persist.tile([P, P], BF16)
            wo_sb = persist.tile([P, 2, HIDDEN], BF16)
            bias_sb = persist.tile([P, KB], F32)
            bdve_sb = persist.tile([P, KB], F32)

            # critical-path inputs first (wkv + xt0 feed the first
            # chain); the wo bulk last - needed only by o_proj
            nc.gpsimd.dma_start(out=id_sb, in_=identT[:, :])
            nc.gpsimd.dma_start(out=wkv_sb, in_=wkvT[:, :, :])
            nc.gpsimd.dma_start(out=bias_sb, in_=bias2d[:, :])
            nc.gpsimd.dma_start(out=bdve_sb, in_=bdve[:, :])
            nc.gpsimd.dma_start(out=vaug[:, :, D:D + 1], in_=ones2d[:, :])
            nc.scalar.dma_start(out=xts[0][:, 0:4, :], in_=xT[:, 0, 0:4, :])
            nc.scalar.dma_start(out=xts[0][:, 4:8, :], in_=xT[:, 0, 4:8, :])
            nc.sync.dma_start(out=wq_sb, in_=wqT[:, :, :])
            nc.sync.dma_start(out=xts[1][:, 0:4, :], in_=xT[:, 1, 0:4, :])
            nc.gpsimd.dma_start(out=xts[1][:, 4:8, :], in_=xT[:, 1, 4:8, :])
            nc.sync.dma_start(out=xts[2][:, 0:4, :], in_=xT[:, 2, 0:4, :])
            nc.gpsimd.dma_start(out=xts[2][:, 4:8, :], in_=xT[:, 2, 4:8, :])
            nc.scalar.dma_start(out=xts[3][:, 0:4, :], in_=xT[:, 3, 0:4, :])
            nc.sync.dma_start(out=xts[3][:, 4:8, :], in_=xT[:, 3, 4:8, :])
            nc.gpsimd.dma_start(out=wo_sb, in_=woT[:, :, :])
            warmup = persist.tile([P, 1], F32)
            nc.scalar.activation(warmup, bias_sb[:, 0:1],
                                 mybir.ActivationFunctionType.Exp)

            # ---- projections (j-major, start as soon as xt_j lands) ----
            with tc.tile_pool(name="projq_ps", bufs=5, space="PSUM") as ppsq, \
                 tc.tile_pool(name="projv_ps", bufs=3, space="PSUM") as ppsv:
                # HAM warmup: junk matmuls on the early identity tile keep
                # the PE busy while the big input DMAs stream, so the real
                # chains run at 2.4 GHz from the start
                for w in range(36):
                    jp = ppsq.tile([P, P], F32, tag="pq", name="jp")
                    nc.tensor.matmul(jp, lhsT=id_sb[:, 0:P],
                                     rhs=id_sb[:, 0:P],
                                     start=True, stop=True)
                for j in range(QC):
                    pkv = ppsq.tile([P, 512], F32, tag="pq", name="pkv")
                    for kc in range(8):
                        nc.tensor.matmul(
                            pkv, lhsT=wkv_sb[:, kc, :],
                            rhs=xts[j][:, kc, :],
                            start=(kc == 0), stop=(kc == 7))
                    nc.vector.tensor_copy(
                        kt[0:D, j * 512:(j + 1) * 512], pkv[0:D, :])
                    nc.vector.tensor_copy(
                        vt_sb[D:P, j * 512:(j + 1) * 512], pkv[D:P, :])
                    # duplicate k rows to partitions 64:128
                    nc.sync.dma_start(
                        out=kt[D:P, j * 512:(j + 1) * 512],
                        in_=kt[0:D, j * 512:(j + 1) * 512])
                    for sc in range(4 * j, 4 * j + 4):
                        pv = ppsv.tile([P, D], BF16, tag="pv")
                        nc.tensor.transpose(
                            pv, vt_sb[D:P, sc * P:(sc + 1) * P],
                            id_sb[D:P, D:P])
                        nc.vector.tensor_copy(vaug[:, sc, 0:D], pv)
                    for pair in range(2):
                        pq = ppsq.tile([P, 512], F32, tag="pq", name="pq")
                        for kc in range(8):
                            nc.tensor.matmul(
                                pq, lhsT=wq_sb[:, kc,
                                               pair * P:(pair + 1) * P],
                                rhs=xts[j][:, kc, :],
                                start=(kc == 0), stop=(kc == 7))
                        if pair == 0:
                            nc.vector.tensor_copy(
                                qt[:, 0, j * 512:(j + 1) * 512], pq)
                        else:
                            nc.scalar.copy(
                                qt[:, 1, j * 512:(j + 1) * 512], pq)

            # ---- attention (software-pipelined, LAG units) ----
            with tc.tile_pool(name="sc_ps", bufs=2, space="PSUM") as scp, \
                 tc.tile_pool(name="att_ps", bufs=2, space="PSUM") as attp, \
                 tc.tile_pool(name="exp_sb", bufs=6) as expp, \
                 tc.tile_pool(name="norm_sb", bufs=4) as normp:
                units = [(j, pair, kb) for j in range(QC)
                         for pair in range(2) for kb in range(KB)]
                LAG = 5
                att_tiles = {}
                ex_store = {}

                norm_stages = []  # (due_u, closure)

                def emit_norm(j, pair, attA, attB, u0):
                    deng = nc.sync if j == QC - 1 else nc.gpsimd
                    st = {}
                    for h01, attP in ((0, attA), (1, attB)):
                        tmp = normp.tile([D + 1, 512], F32, tag="tmp")
                        nc.vector.tensor_copy(tmp, attP)  # frees att bank
                        ds = normp.tile([D, 8], F32, tag="ds")
                        deng.dma_start(out=ds, in_=tmp[D:D + 1, :])
                        st[h01] = (tmp, ds)

                    def s1():
                        for h01 in (0, 1):
                            tmp, ds = st[h01]
                            rs = normp.tile([D, 8], F32, tag="rs")
                            nc.vector.reciprocal(out=rs, in_=ds)
                            wdma = deng.dma_start(
                                out=bounce[j, pair, h01, :], in_=rs)
                            st[h01] = (tmp, wdma)

                    def s2():
                        for h01 in (0, 1):
                            tmp, wdma = st[h01]
                            bc = normp.tile([D, 1, 512], F32, tag="bc")
                            rdma = deng.dma_start(
                                out=bc,
                                in_=bounce[j, pair,
                                           h01, :].partition_broadcast(D))
                            add_dep_helper(rdma.ins, wdma.ins,
                                           reason="bounce RAW")
                            st[h01] = (tmp, bc)

                    def s3():
                        for h01 in (0, 1):
                            tmp, bc = st[h01]
                            if h01 == 0:
                                nc.vector.tensor_mul(
                                    attnT_js[j][0:D, pair, :],
                                    tmp[0:D, :], bc[:, 0, :])
                            else:
                                nt = normp.tile([D, 512], BF16, tag="nt")
                                nc.vector.tensor_mul(nt, tmp[0:D, :],
                                                     bc[:, 0, :])
                                deng.dma_start(
                                    out=attnT_js[j][D:P, pair, :],
                                    in_=nt)

                    norm_stages.append((u0 + 2, s1))
                    norm_stages.append((u0 + 4, s2))
                    norm_stages.append((u0 + 6, s3))

                for u in range(len(units) + LAG):
                    if u < len(units):
                        j, pair, kb = units[u]
                        if kb == 0:
                            attA_t = attp.tile([D + 1, 512], F32,
                                               tag="attA", name=f"attA_{u}")
                            attB_t = attp.tile([D + 1, 512], F32,
                                               tag="attB", name=f"attB_{u}")
                            att_tiles[(j, pair)] = (attA_t, attB_t)
                        sc = scp.tile([P, 1024], F32, tag="sc")
                        nc.tensor.matmul(
                            sc[:, 0:512],
                            lhsT=kt[0:D, kb * P:(kb + 1) * P],
                            rhs=qt[0:D, pair, j * 512:(j + 1) * 512],
                            start=True, stop=True)
                        nc.tensor.matmul(
                            sc[:, 512:1024],
                            lhsT=kt[D:P, kb * P:(kb + 1) * P],
                            rhs=qt[D:P, pair, j * 512:(j + 1) * 512],
                            start=True, stop=True)
                        ex = expp.tile([P, 1024], BF16, tag="ex")
                        if kb in DVE_KBS:
                            nc.vector.tensor_scalar(
                                out=ex.bitcast(I16), in0=sc,
                                scalar1=SCH_A,
                                scalar2=bdve_sb[:, kb:kb + 1],
                                op0=mybir.AluOpType.mult,
                                op1=mybir.AluOpType.add)
                        else:
                            nc.scalar.activation(
                                ex, sc, mybir.ActivationFunctionType.Exp,
                                bias=bias_sb[:, kb:kb + 1], scale=1.0)
                        ex_store[u] = ex
                    if u >= LAG:
                        j2, pair2, kb2 = units[u - LAG]
                        attA, attB = att_tiles[(j2, pair2)]
                        ex2 = ex_store.pop(u - LAG)
                        nc.tensor.matmul(
                            attA, lhsT=vaug[:, kb2, :], rhs=ex2[:, 0:512],
                            start=(kb2 == 0), stop=(kb2 == KB - 1))
                        nc.tensor.matmul(
                            attB, lhsT=vaug[:, kb2, :], rhs=ex2[:, 512:1024],
                            start=(kb2 == 0), stop=(kb2 == KB - 1))
                        if kb2 == KB - 1:
                            emit_norm(j2, pair2, attA, attB, u)
                            del att_tiles[(j2, pair2)]
                    while norm_stages and norm_stages[0][0] <= u:
                        norm_stages.pop(0)[1]()
                while norm_stages:
                    norm_stages.pop(0)[1]()

            # ---- o_proj (row-parallel partial, separate phase) ----
            with tc.tile_pool(name="o_ps", bufs=4, space="PSUM") as ops, \
                 tc.tile_pool(name="o_sb", bufs=6) as osb:
                for sc in range(KB):
                    ot = osb.tile([P, 1024], BF16, tag="ot")
                    for n in range(2):
                        po = ops.tile([P, 512], F32, tag="po")
                        for t in range(2):
                            nc.tensor.matmul(
                                po,
                                lhsT=attnT_js[sc // 4][:, t,
                                                       (sc % 4) * P:
                                                       (sc % 4 + 1) * P],
                                rhs=wo_sb[:, t, n * 512:(n + 1) * 512],
                                start=(t == 0), stop=(t == 1))
                        if n == 0:
                            nc.vector.tensor_copy(ot[:, 0:512], po)
                        else:
                            nc.scalar.copy(ot[:, 512:1024], po)
                    # keep the sync ring clear for the last qchunk's
                    # norm round-trip
                    eng = (nc.scalar, nc.gpsimd)[sc % 2]
                    eng.dma_start(
                        out=out[sc * P:(sc + 1) * P, :], in_=ot)

    nc.finalize()
    return nc


def make_in_maps(hidden_states, attention_mask, wq, wk, wv, wo):
    scale = D ** -0.5
    wq_s = (wq * scale).astype(np.float32)
    in_maps = []
    for c in range(NCORES):
        b = c // 4
        g = c % 4
        h0 = g * HEADS_PER_CORE * D
        xTt = np.asarray(hidden_states[b]).T
        xTc = np.ascontiguousarray(
            xTt.reshape(8, P, QC, 512).transpose(1, 2, 0, 3))
        wqTc = np.ascontiguousarray(
            wq_s[h0:h0 + 256, :].T.reshape(8, P, 256).transpose(1, 0, 2))
        wkvT = np.concatenate([wk.T, wv.T], axis=1).astype(np.float32)
        wkvTc = np.ascontiguousarray(
            wkvT.reshape(8, P, P).transpose(1, 0, 2))
        woTc = np.ascontiguousarray(
            wo[:, h0:h0 + 256].T.reshape(2, P, HIDDEN).transpose(1, 0, 2))
        bias = ((1.0 - attention_mask[b]) * -1e30).astype(np.float32)
        bias2d = np.ascontiguousarray(bias.reshape(KB, P).T)
        bdve = np.maximum(SCH_B + SCH_A * bias2d.astype(np.float64),
                          -60000.0).astype(np.float32)
        in_maps.append({
            "xT": xTc.astype(ml_dtypes.bfloat16),
            "wqT": wqTc.astype(ml_dtypes.bfloat16),
            "wkvT": wkvTc.astype(ml_dtypes.bfloat16),
            "identT": np.eye(P).astype(ml_dtypes.bfloat16),
            "woT": woTc.astype(ml_dtypes.bfloat16),
            "bias2d": bias2d,
            "bdve": bdve,
            "ones2d": np.ones((P, KB), dtype=ml_dtypes.bfloat16),
        })
    return in_maps


def run(inputs, trace=False, trace_cores=None):
    if "nc" not in _CACHE:
        _CACHE["nc"] = build_kernel()
    nc = _CACHE["nc"]
    in_maps = make_in_maps(**inputs)
    res = run_bass_kernel_spmd(
        nc, in_maps, list(range(NCORES)), trace=trace,
        trace_cores=trace_cores)
    parts = [res.results[c]["out"] for c in range(NCORES)]
    full = np.empty((B, S, HIDDEN), dtype=np.float32)
    for b in range(B):
        acc = np.zeros((S, HIDDEN), dtype=np.float64)
        for g in range(4):
            acc += parts[4 * b + g]
        full[b] = acc.astype(np.float32)
    return full, res


def kernel(hidden_states, attention_mask, wq, wk, wv, wo):
    full, _ = run(dict(hidden_states=np.asarray(hidden_states),
                       attention_mask=np.asarray(attention_mask),
                       wq=np.asarray(wq), wk=np.asarray(wk),
                       wv=np.asarray(wv), wo=np.asarray(wo)))
    return full


# revision 65
# speedup vs baseline: 1.0097x; 1.0097x over previous
"""MQA attention (16 Q heads, 1 KV head) on 8 trn2 NeuronCores.

Sharding: data-parallel on batch (2) x tensor-parallel on Q heads (4 per
core). Each core computes K/V for its batch, attention for its 4 heads,
and a row-parallel o_proj partial; the host sums the 4 partials per batch.

Per-core layout strategy: all matmul contractions on partitions, all
matmul operands bf16.
  front end: inputs land as a few big partition-major DMAs (4-32KB
     contiguous runs per partition) with critical-path priority - wkv
     first on the gpsimd ring, xt0 halves on the scalar ring, wq first
     on the sync ring, the wo bulk last; HAM-warmup junk matmuls on the
     early identity tile keep the PE active during the DMA wait so the
     real chains run at 2.4 GHz from the first tile
  kv proj packed into one chain (k -> PSUM rows 0:64, v -> 64:128); k
     duplicated to partitions 64:128 by one SBUF-SBUF DMA per chunk so
     the two row-tiled score matmuls see k on both halves; v transposed
     to [keys, dim] blocks by PE transposes from partitions 64:128
  qT = wqT.T @ xT -> [256, 2048] as 2 head-pair tiles [128, 2048]
  scoresT [k, q] per (pair, qchunk, kblock) in PSUM [128, 1024]: two
     K=64 matmuls on disjoint partition halves (head-even on array rows
     0:63, head-odd on 64:127) which partially overlap on the PE
  exp: ScalarE table-exp with per-key bias for 11/16 key blocks; the
     other 5 on DVE via the Schraudolph bit-trick (i16 = a*s + b,
     bitcast to bf16, ~2% rms) so the ACT engine keeps pace
  PV: lhsT = [v | ones] bf16 [128, 65] -> attn_outT [64, q] + denom row
  normalize: reciprocal + DRAM-bounce partition-broadcast + DVE multiply,
     staged across later units so the in-order DVE queue never blocks
  o_proj: out[q, hidden] partial = attnT.T @ woT (bf16), K=256, as a
     separate phase; output DMAs alternate scalar/gpsimd rings keeping
     sync clear for the last qchunk's norm round-trip
"""
import sys

sys.path.insert(0, "/opt/trn_rl_repo")

import ml_dtypes
import numpy as np

import concourse.bass as bass
import concourse.bacc as bacc
import concourse.tile as tile
from concourse import mybir
from concourse.bass_utils import run_bass_kernel_spmd
from concourse.tile_rust import add_dep_helper

HIDDEN = 1024
NH = 16
D = 64
B = 2
S = 2048
NCORES = 8
HEADS_PER_CORE = 4
KB = S // 128   # 16 key blocks
QC = S // 512   # 4 query chunks
P = 128

F32 = mybir.dt.float32
F32R = mybir.dt.float32r
BF16 = mybir.dt.bfloat16
I16 = mybir.dt.int16

DVE_KBS = (2, 5, 8, 11, 14)
SCH_A = 128.0 / float(np.log(2.0))
SCH_B = 16256.0 - 0.0397 * SCH_A + 0.5

_CACHE = {}


def build_kernel():
    nc = bacc.Bacc("TRN2", target_bir_lowering=False, debug=False,
                   num_devices=NCORES)

    xT = nc.dram_tensor("xT", [P, QC, 8, 512], BF16, kind="ExternalInput")
    wqT = nc.dram_tensor("wqT", [P, 8, 256], BF16, kind="ExternalInput")
    wkvT = nc.dram_tensor("wkvT", [P, 8, 128], BF16, kind="ExternalInput")
    identT = nc.dram_tensor("identT", [P, P], BF16, kind="ExternalInput")
    woT = nc.dram_tensor("woT", [P, 2, HIDDEN], BF16, kind="ExternalInput")
    bias2d = nc.dram_tensor("bias2d", [P, KB], F32, kind="ExternalInput")
    bdve = nc.dram_tensor("bdve", [P, KB], F32, kind="ExternalInput")
    ones2d = nc.dram_tensor("ones2d", [P, KB], BF16, kind="ExternalInput")
    out = nc.dram_tensor("out", [S, HIDDEN], BF16, kind="ExternalOutput")
    bounce = nc.dram_tensor("bounce", [QC, 2, 2, 512], F32)

    with tile.TileContext(nc) as tc:
        with tc.tile_pool(name="persist", bufs=1) as persist:
            xts = [persist.tile([P, 8, 512], BF16, name=f"xt{jj}")
                   for jj in range(QC)]  # per-qchunk xT tiles
            qt = persist.tile([P, 2, S], BF16)          # qT head pairs
            kt = persist.tile([P, S], BF16)             # kT dup both halves
            vaug = persist.tile([P, KB, D + 1], BF16)   # [v | ones]
            attnT_js = [persist.tile([P, 2, 512], BF16, name=f"attnT{jj}")
                        for jj in range(QC)]  # per-j normalized attnT
            wq_sb = persist.tile([P, 8, 256], BF16)
            wkv_sb = persist.tile([P, 8, 128], BF16)
            vt_sb = persist.tile([P, S], BF16)
            id_sb = persist.tile([P, P], BF16)
            wo_sb = persist.tile([P, 2, HIDDEN], BF16)
            bias_sb = persist.tile([P, KB], F32)
            bdve_sb = persist.tile([P, KB], F32)

            # critical-path inputs first (wkv + xt0 feed the first
            # chain); the wo bulk last - needed only by o_proj
            nc.gpsimd.dma_start(out=id_sb, in_=identT[:, :])
            nc.gpsimd.dma_start(out=wkv_sb, in_=wkvT[:, :, :])
            nc.gpsimd.dma_start(out=bias_sb, in_=bias2d[:, :])
            nc.gpsimd.dma_start(out=bdve_sb, in_=bdve[:, :])
            nc.gpsimd.dma_start(out=vaug[:, :, D:D + 1], in_=ones2d[:, :])
            nc.scalar.dma_start(out=xts[0][:, 0:4, :], in_=xT[:, 0, 0:4, :])
            nc.scalar.dma_start(out=xts[0][:, 4:8, :], in_=xT[:, 0, 4:8, :])
            nc.sync.dma_start(out=wq_sb, in_=wqT[:, :, :])
            nc.sync.dma_start(out=xts[1][:, 0:4, :], in_=xT[:, 1, 0:4, :])
            nc.gpsimd.dma_start(out=xts[1][:, 4:8, :], in_=xT[:, 1, 4:8, :])
            nc.sync.dma_start(out=xts[2][:, 0:4, :], in_=xT[:, 2, 0:4, :])
            nc.gpsimd.dma_start(out=xts[2][:, 4:8, :], in_=xT[:, 2, 4:8, :])
            nc.scalar.dma_start(out=xts[3][:, 0:4, :], in_=xT[:, 3, 0:4, :])
            nc.sync.dma_start(out=xts[3][:, 4:8, :], in_=xT[:, 3, 4:8, :])
            nc.gpsimd.dma_start(out=wo_sb, in_=woT[:, :, :])
            warmup = persist.tile([P, 1], F32)
            nc.scalar.activation(warmup, bias_sb[:, 0:1],
                                 mybir.ActivationFunctionType.Exp)

            # ---- projections (j-major, start as soon as xt_j lands) ----
            with tc.tile_pool(name="projq_ps", bufs=4, space="PSUM") as ppsq, \
                 tc.tile_pool(name="projv_ps", bufs=2, space="PSUM") as ppsv:
                # HAM warmup: junk matmuls on the early identity tile keep
                # the PE busy while the big input DMAs stream, so the real
                # chains run at 2.4 GHz from the start
                for w in range(36):
                    jp = ppsq.tile([P, P], F32, tag="pq", name="jp")
                    nc.tensor.matmul(jp, lhsT=id_sb[:, 0:P],
                                     rhs=id_sb[:, 0:P],
                                     start=True, stop=True)
                for j in range(QC):
                    pkv = ppsq.tile([P, 512], F32, tag="pq", name="pkv")
                    for kc in range(8):
                        nc.tensor.matmul(
                            pkv, lhsT=wkv_sb[:, kc, :],
                            rhs=xts[j][:, kc, :],
                            start=(kc == 0), stop=(kc == 7))
                    nc.vector.tensor_copy(
                        kt[0:D, j * 512:(j + 1) * 512], pkv[0:D, :])
                    nc.vector.tensor_copy(
                        vt_sb[D:P, j * 512:(j + 1) * 512], pkv[D:P, :])
                    # duplicate k rows to partitions 64:128
                    nc.sync.dma_start(
                        out=kt[D:P, j * 512:(j + 1) * 512],
                        in_=kt[0:D, j * 512:(j + 1) * 512])
                    for sc in range(4 * j, 4 * j + 4):
                        pv = ppsv.tile([P, D], BF16, tag="pv")
                        nc.tensor.transpose(
                            pv, vt_sb[D:P, sc * P:(sc + 1) * P],
                            id_sb[D:P, D:P])
                        nc.vector.tensor_copy(vaug[:, sc, 0:D], pv)
                    for pair in range(2):
                        pq = ppsq.tile([P, 512], F32, tag="pq", name="pq")
                        for kc in range(8):
                            nc.tensor.matmul(
                                pq, lhsT=wq_sb[:, kc,
                                               pair * P:(pair + 1) * P],
                                rhs=xts[j][:, kc, :],
                                start=(kc == 0), stop=(kc == 7))
                        if pair == 0:
                            nc.vector.tensor_copy(
                                qt[:, 0, j * 512:(j + 1) * 512], pq)
                        else:
                            nc.scalar.copy(
                                qt[:, 1, j * 512:(j + 1) * 512], pq)

            # ---- attention (software-pipelined, LAG units) ----
            with tc.tile_pool(name="sc_ps", bufs=2, space="PSUM") as scp, \
                 tc.tile_pool(name="att_ps", bufs=1, space="PSUM") as attp, \
                 tc.tile_pool(name="o_ps", bufs=2, space="PSUM") as ops, \
                 tc.tile_pool(name="exp_sb", bufs=6) as expp, \
                 tc.tile_pool(name="norm_sb", bufs=4) as normp, \
                 tc.tile_pool(name="o_sb", bufs=4) as osb:
                units = [(j, pair, kb) for j in range(QC)
                         for pair in range(2) for kb in range(KB)]
                LAG = 5
                att_tiles = {}
                ex_store = {}

                norm_stages = []  # (due_u, closure)

                def emit_norm(j, pair, attA, attB, u0):
                    deng = nc.sync if j == QC - 1 else nc.gpsimd
                    st = {}
                    for h01, attP in ((0, attA), (1, attB)):
                        tmp = normp.tile([D + 1, 512], F32, tag="tmp")
                        nc.vector.tensor_copy(tmp, attP)  # frees att bank
                        ds = normp.tile([D, 8], F32, tag="ds")
                        deng.dma_start(out=ds, in_=tmp[D:D + 1, :])
                        st[h01] = (tmp, ds)

                    def s1():
                        for h01 in (0, 1):
                            tmp, ds = st[h01]
                            rs = normp.tile([D, 8], F32, tag="rs")
                            nc.vector.reciprocal(out=rs, in_=ds)
                            wdma = deng.dma_start(
                                out=bounce[j, pair, h01, :], in_=rs)
                            st[h01] = (tmp, wdma)

                    def s2():
                        for h01 in (0, 1):
                            tmp, wdma = st[h01]
                            bc = normp.tile([D, 1, 512], F32, tag="bc")
                            rdma = deng.dma_start(
                                out=bc,
                                in_=bounce[j, pair,
                                           h01, :].partition_broadcast(D))
                            add_dep_helper(rdma.ins, wdma.ins,
                                           reason="bounce RAW")
                            st[h01] = (tmp, bc)

                    def s3():
                        for h01 in (0, 1):
                            tmp, bc = st[h01]
                            if h01 == 0:
                                nc.vector.tensor_mul(
                                    attnT_js[j][0:D, pair, :],
                                    tmp[0:D, :], bc[:, 0, :])
                            else:
                                nt = normp.tile([D, 512], BF16, tag="nt")
                                nc.vector.tensor_mul(nt, tmp[0:D, :],
                                                     bc[:, 0, :])
                                deng.dma_start(
                                    out=attnT_js[j][D:P, pair, :],
                                    in_=nt)

                    norm_stages.append((u0 + 2, s1))
                    norm_stages.append((u0 + 4, s2))
                    norm_stages.append((u0 + 6, s3))

                for u in range(len(units) + LAG):
                    if u < len(units):
                        j, pair, kb = units[u]
                        sc = scp.tile([P, 1024], F32, tag="sc")
                        nc.tensor.matmul(
                            sc[:, 0:512],
                            lhsT=kt[0:D, kb * P:(kb + 1) * P],
                            rhs=qt[0:D, pair, j * 512:(j + 1) * 512],
                            start=True, stop=True)
                        nc.tensor.matmul(
                            sc[:, 512:1024],
                            lhsT=kt[D:P, kb * P:(kb + 1) * P],
                            rhs=qt[D:P, pair, j * 512:(j + 1) * 512],
                            start=True, stop=True)
                        ex = expp.tile([P, 1024], BF16, tag="ex")
                        if kb in DVE_KBS:
                            nc.vector.tensor_scalar(
                                out=ex.bitcast(I16), in0=sc,
                                scalar1=SCH_A,
                                scalar2=bdve_sb[:, kb:kb + 1],
                                op0=mybir.AluOpType.mult,
                                op1=mybir.AluOpType.add)
                        else:
                            nc.scalar.activation(
                                ex, sc, mybir.ActivationFunctionType.Exp,
                                bias=bias_sb[:, kb:kb + 1], scale=1.0)
                        ex_store[u] = ex
                    if u >= LAG:
                        j2, pair2, kb2 = units[u - LAG]
                        if kb2 == 0:
                            # allocate at first write: with bufs=1 the WAR
                            # reuse then lands after the previous group's
                            # final PV and norm copy
                            attA_t = attp.tile([D + 1, 512], F32,
                                               tag="attA",
                                               name=f"attA_{u - LAG}")
                            attB_t = attp.tile([D + 1, 512], F32,
                                               tag="attB",
                                               name=f"attB_{u - LAG}")
                            att_tiles[(j2, pair2)] = (attA_t, attB_t)
                        attA, attB = att_tiles[(j2, pair2)]
                        ex2 = ex_store.pop(u - LAG)
                        nc.tensor.matmul(
                            attA, lhsT=vaug[:, kb2, :], rhs=ex2[:, 0:512],
                            start=(kb2 == 0), stop=(kb2 == KB - 1))
                        nc.tensor.matmul(
                            attB, lhsT=vaug[:, kb2, :], rhs=ex2[:, 512:1024],
                            start=(kb2 == 0), stop=(kb2 == KB - 1))
                        if kb2 == KB - 1:
                            emit_norm(j2, pair2, attA, attB, u)
                            del att_tiles[(j2, pair2)]
                    while norm_stages and norm_stages[0][0] <= u:
                        norm_stages.pop(0)[1]()
                while norm_stages:
                    norm_stages.pop(0)[1]()

                # ---- o_proj (row-parallel partial; dedicated pool lets
                # the scheduler overlap it with the attention tail) ----
                for sc in range(KB):
                    ot = osb.tile([P, 1024], BF16, tag="ot")
                    for n in range(2):
                        po = ops.tile([P, 512], F32, tag="po")
                        for t in range(2):
                            nc.tensor.matmul(
                                po,
                                lhsT=attnT_js[sc // 4][:, t,
                                                       (sc % 4) * P:
                                                       (sc % 4 + 1) * P],
                                rhs=wo_sb[:, t, n * 512:(n + 1) * 512],
                                start=(t == 0), stop=(t == 1))
                        if n == 0:
                            nc.vector.tensor_copy(ot[:, 0:512], po)
                        else:
                            nc.scalar.copy(ot[:, 512:1024], po)
                    # keep the sync ring clear for the last qchunk's
                    # norm round-trip
                    eng = (nc.scalar, nc.gpsimd)[sc % 2]
                    eng.dma_start(
                        out=out[sc * P:(sc + 1) * P, :], in_=ot)

    nc.finalize()
    return nc


def make_in_maps(hidden_states, attention_mask, wq, wk, wv, wo):
    scale = D ** -0.5
    wq_s = (wq * scale).astype(np.float32)
    in_maps = []
    for c in range(NCORES):
        b = c // 4
        g = c % 4
        h0 = g * HEADS_PER_CORE * D
        xTt = np.asarray(hidden_states[b]).T
        xTc = np.ascontiguousarray(
            xTt.reshape(8, P, QC, 512).transpose(1, 2, 0, 3))
        wqTc = np.ascontiguousarray(
            wq_s[h0:h0 + 256, :].T.reshape(8, P, 256).transpose(1, 0, 2))
        wkvT = np.concatenate([wk.T, wv.T], axis=1).astype(np.float32)
        wkvTc = np.ascontiguousarray(
            wkvT.reshape(8, P, P).transpose(1, 0, 2))
        woTc = np.ascontiguousarray(
            wo[:, h0:h0 + 256].T.reshape(2, P, HIDDEN).transpose(1, 0, 2))
        bias = ((1.0 - attention_mask[b]) * -1e30).astype(np.float32)
        bias2d = np.ascontiguousarray(bias.reshape(KB, P).T)
        bdve = np.maximum(SCH_B + SCH_A * bias2d.astype(np.float64),
                          -60000.0).astype(np.float32)
        in_maps.append({
            "xT": xTc.astype(ml_dtypes.bfloat16),
            "wqT": wqTc.astype(ml_dtypes.bfloat16),
            "wkvT": wkvTc.astype(ml_dtypes.bfloat16),
            "identT": np.eye(P).astype(ml_dtypes.bfloat16),
            "woT": woTc.astype(ml_dtypes.bfloat16),
            "bias2d": bias2d,
            "bdve": bdve,
            "ones2d": np.ones((P, KB), dtype=ml_dtypes.bfloat16),
        })
    return in_maps


def run(inputs, trace=False, trace_cores=None):
    if "nc" not in _CACHE:
        _CACHE["nc"] = build_kernel()
    nc = _CACHE["nc"]
    in_maps = make_in_maps(**inputs)
    res = run_bass_kernel_spmd(
        nc, in_maps, list(range(NCORES)), trace=trace,
        trace_cores=trace_cores)
    parts = [res.results[c]["out"] for c in range(NCORES)]
    full = np.empty((B, S, HIDDEN), dtype=np.float32)
    for b in range(B):
        acc = np.zeros((S, HIDDEN), dtype=np.float64)
        for g in range(4):
            acc += parts[4 * b + g]
        full[b] = acc.astype(np.float32)
    return full, res


def kernel(hidden_states, attention_mask, wq, wk, wv, wo):
    full, _ = run(dict(hidden_states=np.asarray(hidden_states),
                       attention_mask=np.asarray(attention_mask),
                       wq=np.asarray(wq), wk=np.asarray(wk),
                       wv=np.asarray(wv), wo=np.asarray(wo)))
    return full
